# revision 18
# baseline (speedup 1.0000x reference)
"""Trainium2 Bass kernel for nn_Meta_LearnerFF (gnn_message_passing).

Computes forward + analytic gradients of:
    f1 = relu(W1 @ x + b1)            per column  [H, W]
    F  = relu(W2 @ f1 + b2)           per column  [H, W]
    u  = (Wi*mask) @ concat(F.flat, hidden) + bi  [H]
    h_t = tanh(u);  s = sum(h_t);  y = pred . h_t
with the per-target-column stop_gradient mixing pattern, which makes all
gradients analytic:
    t      = 1 - h_t^2
    gbi    = t
    gWi    = t[:,None] * mask * feat[None,:]
    ghid   = t * Wi[:,HW+m]*mask[:,HW+m]   (diagonal)
    gF     = t[h] * Wi[h, h*W+w]           (own block, mask forced 1)
    gz2    = gF * (F>0);  gb2 = gz2;  gW2 = gz2 x f1;  gf1 = gz2 @ W2
    gz1    = gf1 * (f1>0); gb1 = gz1;  gW1 = gz1 x x

Sharding: column axis (H=512) split 64 rows/core across 8 NeuronCores.
Each core computes its rows' forward features, an AllGather shares the
small [512,10] feature matrix, then each core computes its slice of u,
h_t and all gradient slices.  Since t = 1-h_t^2 gates every gradient,
all t-independent gradient factors are precomputed before/during the
collective; the post-tanh critical path is a handful of per-partition
scalar multiplies.  Host glue only reshapes/slices inputs and
concatenates outputs (plus an 8-way partial sum for the scalar y).
"""

import os
import sys

import numpy as np

for _p in ("/opt/trn_rl_repo", "/root/.axon_site/_ro/trn_rl_repo"):
    if _p not in sys.path and os.path.isdir(_p):
        sys.path.append(_p)

import concourse.bass as bass
import concourse.bacc as bacc
import concourse.mybir as mybir
import concourse.tile as tile
from concourse.bass_utils import run_bass_kernel_spmd

F32 = mybir.dt.float32
ALU = mybir.AluOpType
ACTF = mybir.ActivationFunctionType

H = 512          # hidden_nodes (columns)
W = 10           # column width
DIN = 4096       # input dim
HW = H * W       # 5120
DTOT = HW + H    # 5632
NCORES = 8
RPC = H // NCORES           # 64 rows per core
FR = RPC * W                # 640 fc1 rows per core
NT = FR // 128              # 5 tiles of 128 partitions
JH = DTOT // 2              # 2816: i-layer free dim per partition-half

LAST_RESULTS = None  # BassKernelResults of the most recent run (for test.py)


def build_program():
    """Build the single-core SPMD Bass/Tile program (same NEFF on 8 cores)."""
    nc = bacc.Bacc(
        "TRN2", target_bir_lowering=False, debug=False, num_devices=NCORES
    )

    # ---- I/O ----
    xbc_d = nc.dram_tensor("xbc_in", [128, DIN], F32, kind="ExternalInput")
    hid_d = nc.dram_tensor("hid_in", [H], F32, kind="ExternalInput")
    w1_d = nc.dram_tensor("w1_il", [NT, 128, DIN], F32, kind="ExternalInput")
    b1_d = nc.dram_tensor("b1_il", [128, NT], F32, kind="ExternalInput")
    w2_d = nc.dram_tensor("w2_in", [RPC, W * W], F32, kind="ExternalInput")
    w2t_d = nc.dram_tensor("w2t_in", [RPC, W * W], F32, kind="ExternalInput")
    b2_d = nc.dram_tensor("b2_in", [RPC, W], F32, kind="ExternalInput")
    wi_d = nc.dram_tensor("wi_rc", [128, JH], F32, kind="ExternalInput")
    mask_d = nc.dram_tensor("mask_rc", [128, JH], F32, kind="ExternalInput")
    bii_d = nc.dram_tensor("bi_c", [RPC, 1], F32, kind="ExternalInput")
    wio_d = nc.dram_tensor("wi_own", [RPC, W], F32, kind="ExternalInput")
    wdh_d = nc.dram_tensor("wi_dh", [RPC, 1], F32, kind="ExternalInput")
    mdh_d = nc.dram_tensor("mask_dh", [RPC, 1], F32, kind="ExternalInput")
    pred_d = nc.dram_tensor("pred_c", [RPC, 1], F32, kind="ExternalInput")
    s_d = nc.dram_tensor("s_fold", [128, RPC], F32, kind="ExternalInput")
    e_d = nc.dram_tensor("e_bcast", [2, 128], F32, kind="ExternalInput")

    y_o = nc.dram_tensor("y_o", [1, 1], F32, kind="ExternalOutput")
    ht_o = nc.dram_tensor("ht_o", [RPC], F32, kind="ExternalOutput")
    ghid_o = nc.dram_tensor("ghid_o", [RPC], F32, kind="ExternalOutput")
    gw1_o = nc.dram_tensor("gw1_o", [NT, 128, DIN], F32, kind="ExternalOutput")
    gb1_o = nc.dram_tensor("gb1_o", [RPC, W], F32, kind="ExternalOutput")
    gw2_o = nc.dram_tensor("gw2_o", [RPC, W * W], F32, kind="ExternalOutput")
    gb2_o = nc.dram_tensor("gb2_o", [RPC, W], F32, kind="ExternalOutput")
    gwi_o = nc.dram_tensor("gwi_o", [128, JH], F32, kind="ExternalOutput")
    gbi_o = nc.dram_tensor("gbi_o", [RPC], F32, kind="ExternalOutput")

    with tile.TileContext(nc) as tc:
        with (
            tc.tile_pool(name="sing", bufs=1) as sing,
            tc.tile_pool(name="w1p", bufs=3) as w1p,
            tc.tile_pool(name="gwp", bufs=3) as gwp,
            tc.tile_pool(name="big", bufs=1) as big,
            tc.tile_pool(name="psum", bufs=1, space="PSUM") as psum,
            tc.tile_pool(name="dram", bufs=1, space="DRAM") as dram,
        ):
            # ---- x broadcast (pre-replicated on host) ----
            x_bc = sing.tile([128, DIN], F32)
            nc.sync.dma_start(x_bc[:, :], xbc_d.ap())

            # ---- fc1 forward: z1[r] = W1[r,:] . x, rows r = 5p + n ----
            z1sb = sing.tile([128, NT], F32)
            for n in range(NT):
                w1t = w1p.tile([128, DIN], F32, tag="w1t")
                nc.sync.dma_start(w1t[:, :], w1_d.ap()[n])
                scr = gwp.tile([128, DIN], F32, tag="gw")
                nc.vector.scalar_tensor_tensor(
                    scr[:, :],
                    w1t[:, :],
                    1.0,
                    x_bc[:, :],
                    op0=ALU.mult,
                    op1=ALU.mult,
                    accum_out=z1sb[:, n : n + 1],
                )

            # ---- small input loads ----
            b1p = sing.tile([128, NT], F32)
            nc.sync.dma_start(b1p[:, :], b1_d.ap())
            w2_sb = sing.tile([RPC, W * W], F32)
            nc.sync.dma_start(w2_sb[:, :], w2_d.ap())
            w2t_sb = sing.tile([RPC, W * W], F32)
            nc.sync.dma_start(w2t_sb[:, :], w2t_d.ap())
            b2_sb = sing.tile([RPC, W], F32)
            nc.sync.dma_start(b2_sb[:, :], b2_d.ap())
            bii_sb = sing.tile([RPC, 1], F32)
            nc.sync.dma_start(bii_sb[:, :], bii_d.ap())
            wio_sb = sing.tile([RPC, W], F32)
            nc.sync.dma_start(wio_sb[:, :], wio_d.ap())
            wdh_sb = sing.tile([RPC, 1], F32)
            nc.sync.dma_start(wdh_sb[:, :], wdh_d.ap())
            mdh_sb = sing.tile([RPC, 1], F32)
            nc.sync.dma_start(mdh_sb[:, :], mdh_d.ap())
            pred_sb = sing.tile([RPC, 1], F32)
            nc.sync.dma_start(pred_sb[:, :], pred_d.ap())
            s_sb = sing.tile([128, RPC], F32)
            nc.sync.dma_start(s_sb[:, :], s_d.ap())
            ones_sb = sing.tile([RPC, 1], F32)
            nc.vector.memset(ones_sb[:, :], 1.0)
            e_sb = sing.tile([2, 128], F32)
            nc.sync.dma_start(e_sb[:, :], e_d.ap())
            # feat rows: row0 = F.flat[0:JH], row1 = F.flat[JH:HW] ++ hidden.
            # hidden part has no AG dependency, prefetch early.
            feat2 = sing.tile([2, JH], F32)
            nc.sync.dma_start(feat2[1:2, HW - JH : JH], hid_d.ap())

            # ---- fc2 forward ----
            zb = sing.tile([128, NT], F32)
            nc.vector.tensor_add(zb[:, :], z1sb[:, :], b1p[:, :])
            f1sb = sing.tile([128, NT], F32)
            nc.vector.tensor_scalar_max(f1sb[:, :], zb[:, :], 0.0)
            # repartition [128,5] (r=5p+n) -> [64,10] (r=10h+w); same linear order
            f1hw = sing.tile([RPC, W], F32)
            nc.sync.dma_start(f1hw[:, :], f1sb[:, :])
            drelu1 = sing.tile([RPC, W], F32)
            nc.vector.tensor_scalar(
                drelu1[:, :], f1hw[:, :], 0.0, None, ALU.is_gt
            )
            f1_bv = f1hw[:, :].unsqueeze(1).broadcast_to([RPC, W, W])
            prod2 = sing.tile([RPC, W * W], F32)
            p2v = prod2[:, :].rearrange("h (v w) -> h v w", w=W)
            nc.vector.tensor_mul(
                p2v, w2_sb[:, :].rearrange("h (v w) -> h v w", w=W), f1_bv
            )
            z2 = sing.tile([RPC, W], F32)
            nc.vector.reduce_sum(z2[:, :], p2v, axis=mybir.AxisListType.X)
            z2b = sing.tile([RPC, W], F32)
            nc.vector.tensor_add(z2b[:, :], z2[:, :], b2_sb[:, :])
            f2 = sing.tile([RPC, W], F32)
            nc.vector.tensor_scalar_max(f2[:, :], z2b[:, :], 0.0)

            # ---- t-independent backward factors (run during the AG bubble) ----
            drelu2 = sing.tile([RPC, W], F32)
            nc.vector.tensor_scalar(drelu2[:, :], f2[:, :], 0.0, None, ALU.is_gt)
            wd2 = sing.tile([RPC, W], F32)  # gz2 = t * wd2
            nc.vector.tensor_mul(wd2[:, :], wio_sb[:, :], drelu2[:, :])
            prodt = sing.tile([RPC, W * W], F32)
            ptv = prodt[:, :].rearrange("h (w v) -> h w v", v=W)
            nc.vector.tensor_mul(
                ptv,
                wd2[:, :].unsqueeze(1).broadcast_to([RPC, W, W]),
                w2t_sb[:, :].rearrange("h (w v) -> h w v", v=W),
            )
            pf1 = sing.tile([RPC, W], F32)  # gf1 = t * pf1
            nc.vector.reduce_sum(pf1[:, :], ptv, axis=mybir.AxisListType.X)
            pz1 = sing.tile([RPC, W], F32)  # gz1 = t * pz1
            nc.vector.tensor_mul(pz1[:, :], pf1[:, :], drelu1[:, :])
            # pz1 repartition [64,10] -> [128,5] before t arrives
            pz1p = sing.tile([128, NT], F32)
            nc.sync.dma_start(pz1p[:, :], pz1[:, :])
            pw2 = sing.tile([RPC, W * W], F32)  # gW2 = t * pw2
            nc.vector.tensor_mul(
                pw2[:, :].rearrange("h (v w) -> h v w", w=W),
                wd2[:, :].unsqueeze(2).broadcast_to([RPC, W, W]),
                f1_bv,
            )
            wdm = sing.tile([RPC, 1], F32)  # ghid = t * wdm
            nc.vector.tensor_mul(wdm[:, :], wdh_sb[:, :], mdh_sb[:, :])

            # ---- AllGather F ----
            f_in = dram.tile([RPC, W], F32)
            nc.sync.dma_start(f_in[:, :], f2[:, :])
            f_out = dram.tile([H, W], F32, addr_space="Shared")
            nc.gpsimd.collective_compute(
                "AllGather",
                ALU.bypass,
                replica_groups=[list(range(NCORES))],
                ins=[f_in[:, :].opt()],
                outs=[f_out[:, :].opt()],
            )

            # ---- i-layer weights: loaded during the AG bubble ----
            wi_sb = big.tile([128, JH], F32, tag="bigA")
            nc.sync.dma_start(wi_sb[:, :], wi_d.ap())
            mask_sb = big.tile([128, JH], F32, tag="mask")
            nc.sync.dma_start(mask_sb[:, :], mask_d.ap())
            wm_sb = big.tile([128, JH], F32, tag="bigB")
            nc.vector.tensor_mul(wm_sb[:, :], wi_sb[:, :], mask_sb[:, :])

            # ---- feat broadcast via PE: featb[p,:] = feat2[p>=64 ? 1 : 0, :] ----
            f_fl = f_out[:, :].rearrange("a b -> (a b)")
            nc.sync.dma_start(feat2[0:1, :], f_fl[0:JH])
            nc.sync.dma_start(feat2[1:2, 0 : HW - JH], f_fl[JH:HW])
            featb = psum.tile([128, JH], F32)
            NMM = (JH + 511) // 512
            for q in range(NMM):
                lo = q * 512
                hi = min(JH, lo + 512)
                nc.tensor.matmul(
                    featb[:, lo:hi], e_sb[:, :], feat2[:, lo:hi],
                    start=True, stop=True,
                )

            # ---- u = (Wi*mask) @ feat ; partition p = r + 64s ----
            pu = sing.tile([128, 1], F32)
            scr2 = gwp.tile([128, DIN], F32, tag="gw")
            nc.vector.scalar_tensor_tensor(
                scr2[:, :JH],
                wm_sb[:, :],
                1.0,
                featb[:, :],
                op0=ALU.mult,
                op1=ALU.mult,
                accum_out=pu[:, :],
            )
            u_ps = psum.tile([RPC, 1], F32)
            nc.tensor.matmul(u_ps[:, :], s_sb[:, :], pu[:, :], start=True, stop=True)
            h_sb = sing.tile([RPC, 1], F32)
            nc.scalar.activation(h_sb[:, :], u_ps[:, :], ACTF.Tanh, bias=bii_sb[:, 0:1])
            nc.gpsimd.dma_start(ht_o.ap(), h_sb[:, :])

            # ---- t = 1 - h^2 ----
            hsq = sing.tile([RPC, 1], F32)
            nc.vector.tensor_mul(hsq[:, :], h_sb[:, :], h_sb[:, :])
            t_sb = sing.tile([RPC, 1], F32)
            nc.vector.tensor_scalar(t_sb[:, :], hsq[:, :], -1.0, 1.0, ALU.mult, ALU.add)
            nc.gpsimd.dma_start(gbi_o.ap(), t_sb[:, :])

            # ---- post-t critical path: gz1 -> gz1p -> gW1 ACTs ----
            gz1 = sing.tile([RPC, W], F32)
            nc.vector.tensor_scalar_mul(gz1[:, :], pz1[:, :], t_sb[:, 0:1])
            gz1p = sing.tile([128, NT], F32)
            nc.sync.dma_start(gz1p[:, :], gz1[:, :])
            for n in range(NT):
                gwt = gwp.tile([128, DIN], F32, tag="gw")
                nc.scalar.activation(
                    gwt[:, :], x_bc[:, :], ACTF.Copy, scale=gz1p[:, n : n + 1]
                )
                nc.sync.dma_start(gw1_o.ap()[n], gwt[:, :])

            # ---- remaining small outputs (gpsimd DMAs; off critical path) ----
            nc.gpsimd.dma_start(gb1_o.ap(), gz1[:, :])
            t_il = sing.tile([128, 1], F32)
            nc.vector.tensor_copy(t_il[0:RPC, :], t_sb[:, :])
            nc.gpsimd.dma_start(t_il[RPC:128, :], t_sb[:, :])

            gz2 = sing.tile([RPC, W], F32)
            nc.vector.tensor_scalar_mul(gz2[:, :], wd2[:, :], t_sb[:, 0:1])
            nc.gpsimd.dma_start(gb2_o.ap(), gz2[:, :])
            gw2f = sing.tile([RPC, W * W], F32)
            nc.vector.tensor_scalar_mul(gw2f[:, :], pw2[:, :], t_sb[:, 0:1])
            nc.gpsimd.dma_start(gw2_o.ap(), gw2f[:, :])
            gh = sing.tile([RPC, 1], F32)
            nc.vector.tensor_scalar_mul(gh[:, :], wdm[:, :], t_sb[:, 0:1])
            nc.gpsimd.dma_start(ghid_o.ap(), gh[:, :])

            # ---- y partial = pred_c . h_t ----
            yp = sing.tile([RPC, 1], F32)
            nc.vector.tensor_mul(yp[:, :], pred_sb[:, :], h_sb[:, :])
            y_ps = psum.tile([1, 1], F32)
            nc.tensor.matmul(y_ps[:, :], yp[:, :], ones_sb[:, :], start=True, stop=True)
            y_sb = sing.tile([1, 1], F32)
            nc.vector.tensor_copy(y_sb[:, :], y_ps[:, :])
            nc.gpsimd.dma_start(y_o.ap(), y_sb[:, :])

            # ---- gWi = t * mask * feat (single fused op, off critical path) ----
            gwi_sb = big.tile([128, JH], F32, tag="bigA")
            nc.vector.scalar_tensor_tensor(
                gwi_sb[:, :], mask_sb[:, :], t_il[:, 0:1], featb[:, :],
                op0=ALU.mult, op1=ALU.mult,
            )
            nc.sync.dma_start(gwi_o.ap(), gwi_sb[:, :])

    nc.compile()
    return nc


def shard_inputs(inputs):
    """Full inputs -> per-core in_maps (reshapes/slices/gathers only)."""
    f = lambda a: np.ascontiguousarray(np.asarray(a), dtype=np.float32)
    x = f(inputs["x"])
    hidden = f(inputs["hidden_state"])
    pred = f(inputs["prediction_params"])
    W1 = f(inputs["W1"])
    b1 = f(inputs["b1"])
    W2 = f(inputs["W2"])
    b2 = f(inputs["b2"])
    Wi = f(inputs["Wi"])
    bi = f(inputs["bi"])
    mask = f(inputs["mask"])

    s_fold = np.zeros((128, RPC), np.float32)
    s_fold[np.arange(RPC), np.arange(RPC)] = 1.0
    s_fold[np.arange(RPC) + RPC, np.arange(RPC)] = 1.0
    xbc = np.ascontiguousarray(np.broadcast_to(x, (128, DIN)))
    e_bcast = np.zeros((2, 128), np.float32)
    e_bcast[0, :RPC] = 1.0
    e_bcast[1, RPC:] = 1.0

    in_maps = []
    for c in range(NCORES):
        R = slice(RPC * c, RPC * (c + 1))
        rows = np.arange(RPC * c, RPC * (c + 1))
        w1c = W1[R].reshape(FR, DIN)
        m = {
            "xbc_in": xbc,
            "hid_in": hidden,
            "w1_il": np.ascontiguousarray(
                w1c.reshape(128, NT, DIN).transpose(1, 0, 2)
            ),
            "b1_il": np.ascontiguousarray(b1[R].reshape(128, NT)),
            "w2_in": np.ascontiguousarray(W2[R].reshape(RPC, W * W)),
            "w2t_in": np.ascontiguousarray(
                W2[R].transpose(0, 2, 1).reshape(RPC, W * W)
            ),
            "b2_in": np.ascontiguousarray(b2[R]),
            "wi_rc": np.ascontiguousarray(
                Wi[R].reshape(RPC, 2, JH).transpose(1, 0, 2).reshape(128, JH)
            ),
            "mask_rc": np.ascontiguousarray(
                mask[R].reshape(RPC, 2, JH).transpose(1, 0, 2).reshape(128, JH)
            ),
            "bi_c": np.ascontiguousarray(bi[R].reshape(RPC, 1)),
            "wi_own": np.ascontiguousarray(
                Wi[rows[:, None], W * rows[:, None] + np.arange(W)[None, :]]
            ),
            "wi_dh": np.ascontiguousarray(Wi[rows, HW + rows].reshape(RPC, 1)),
            "mask_dh": np.ascontiguousarray(mask[rows, HW + rows].reshape(RPC, 1)),
            "pred_c": np.ascontiguousarray(pred[R].reshape(RPC, 1)),
            "s_fold": s_fold,
            "e_bcast": e_bcast,
        }
        in_maps.append(m)
    return in_maps


def assemble_outputs(rs):
    """Per-core result dicts -> full output tuple (reference order)."""
    cat = lambda k: np.concatenate([r[k] for r in rs], axis=0)
    y = np.float32(sum(float(r["y_o"][0, 0]) for r in rs))
    h_t = cat("ht_o")
    ghid = cat("ghid_o")
    gW1 = np.concatenate(
        [
            r["gw1_o"].transpose(1, 0, 2).reshape(FR, DIN).reshape(RPC, W, DIN)
            for r in rs
        ],
        axis=0,
    )
    gb1 = cat("gb1_o")
    gW2 = np.concatenate([r["gw2_o"].reshape(RPC, W, W) for r in rs], axis=0)
    gb2 = cat("gb2_o")
    gWi = np.concatenate(
        [
            r["gwi_o"].reshape(2, RPC, JH).transpose(1, 0, 2).reshape(RPC, DTOT)
            for r in rs
        ],
        axis=0,
    )
    gbi = cat("gbi_o")
    return (
        np.asarray(y, np.float32),
        h_t, ghid, gW1, gb1, gW2, gb2, gWi, gbi,
    )


_NC_CACHE = None


def kernel(**inputs):
    global _NC_CACHE, LAST_RESULTS
    if _NC_CACHE is None:
        _NC_CACHE = build_program()
    nc = _NC_CACHE
    in_maps = shard_inputs(inputs)
    tcs = os.environ.get("KERNEL_TRACE_CORES")
    res = run_bass_kernel_spmd(
        nc,
        in_maps,
        core_ids=list(range(NCORES)),
        tmpdir=os.environ.get("KERNEL_TRACE_DIR"),
        trace_cores=[int(c) for c in tcs.split(",")] if tcs else None,
    )
    LAST_RESULTS = res
    return assemble_outputs(res.results)


# revision 24
# speedup vs baseline: 1.1824x; 1.1824x over previous
"""Trainium2 Bass kernel for nn_Meta_LearnerFF (gnn_message_passing).

Computes forward + analytic gradients of:
    f1 = relu(W1 @ x + b1)            per column  [H, W]
    F  = relu(W2 @ f1 + b2)           per column  [H, W]
    u  = (Wi*mask) @ concat(F.flat, hidden) + bi  [H]
    h_t = tanh(u);  s = sum(h_t);  y = pred . h_t
with the per-target-column stop_gradient mixing pattern, which makes all
gradients analytic:
    t      = 1 - h_t^2
    gbi    = t
    gWi    = t[:,None] * mask * feat[None,:]
    ghid   = t * Wi[:,HW+m]*mask[:,HW+m]   (diagonal)
    gF     = t[h] * Wi[h, h*W+w]           (own block, mask forced 1)
    gz2    = gF * (F>0);  gb2 = gz2;  gW2 = gz2 x f1;  gf1 = gz2 @ W2
    gz1    = gf1 * (f1>0); gb1 = gz1;  gW1 = gz1 x x

Sharding: column axis (H=512) split 64 rows/core across 8 NeuronCores.
Each core computes its rows' forward features, an AllGather shares the
small [512,10] feature matrix, then each core computes its slice of u,
h_t and all gradient slices.  Since t = 1-h_t^2 gates every gradient,
all t-independent gradient factors are precomputed before/during the
collective; the post-tanh critical path is a handful of per-partition
scalar multiplies.  Host glue only reshapes/slices inputs and
concatenates outputs (plus an 8-way partial sum for the scalar y).
"""

import os
import sys

import numpy as np

for _p in ("/opt/trn_rl_repo", "/root/.axon_site/_ro/trn_rl_repo"):
    if _p not in sys.path and os.path.isdir(_p):
        sys.path.append(_p)

import concourse.bass as bass
import concourse.bacc as bacc
import concourse.mybir as mybir
import concourse.tile as tile
from concourse.bass_utils import run_bass_kernel_spmd

F32 = mybir.dt.float32
ALU = mybir.AluOpType
ACTF = mybir.ActivationFunctionType

H = 512          # hidden_nodes (columns)
W = 10           # column width
DIN = 4096       # input dim
HW = H * W       # 5120
DTOT = HW + H    # 5632
NCORES = 8
RPC = H // NCORES           # 64 rows per core
FR = RPC * W                # 640 fc1 rows per core
NT = FR // 128              # 5 tiles of 128 partitions
JH = DTOT // 2              # 2816: i-layer free dim per partition-half

LAST_RESULTS = None  # BassKernelResults of the most recent run (for test.py)


def build_program():
    """Build the single-core SPMD Bass/Tile program (same NEFF on 8 cores)."""
    nc = bacc.Bacc(
        "TRN2", target_bir_lowering=False, debug=False, num_devices=NCORES
    )

    # ---- I/O ----
    xbc_d = nc.dram_tensor("xbc_in", [128, DIN], F32, kind="ExternalInput")
    hid_d = nc.dram_tensor("hid_in", [H], F32, kind="ExternalInput")
    w1_d = nc.dram_tensor("w1_il", [NT, 128, DIN], F32, kind="ExternalInput")
    b1_d = nc.dram_tensor("b1_il", [128, NT], F32, kind="ExternalInput")
    w2_d = nc.dram_tensor("w2_in", [RPC, W * W], F32, kind="ExternalInput")
    w2t_d = nc.dram_tensor("w2t_in", [RPC, W * W], F32, kind="ExternalInput")
    b2_d = nc.dram_tensor("b2_in", [RPC, W], F32, kind="ExternalInput")
    wi_d = nc.dram_tensor("wi_rc", [128, JH], F32, kind="ExternalInput")
    mask_d = nc.dram_tensor("mask_rc", [128, JH], F32, kind="ExternalInput")
    bii_d = nc.dram_tensor("bi_c", [RPC, 1], F32, kind="ExternalInput")
    wio_d = nc.dram_tensor("wi_own", [RPC, W], F32, kind="ExternalInput")
    wdh_d = nc.dram_tensor("wi_dh", [RPC, 1], F32, kind="ExternalInput")
    mdh_d = nc.dram_tensor("mask_dh", [RPC, 1], F32, kind="ExternalInput")
    pred_d = nc.dram_tensor("pred_c", [RPC, 1], F32, kind="ExternalInput")
    s_d = nc.dram_tensor("s_fold", [128, RPC], F32, kind="ExternalInput")

    y_o = nc.dram_tensor("y_o", [1, 1], F32, kind="ExternalOutput")
    ht_o = nc.dram_tensor("ht_o", [RPC], F32, kind="ExternalOutput")
    ghid_o = nc.dram_tensor("ghid_o", [RPC], F32, kind="ExternalOutput")
    gw1_o = nc.dram_tensor("gw1_o", [NT, 128, DIN], F32, kind="ExternalOutput")
    gb1_o = nc.dram_tensor("gb1_o", [RPC, W], F32, kind="ExternalOutput")
    gw2_o = nc.dram_tensor("gw2_o", [RPC, W * W], F32, kind="ExternalOutput")
    gb2_o = nc.dram_tensor("gb2_o", [RPC, W], F32, kind="ExternalOutput")
    gwi_o = nc.dram_tensor("gwi_o", [128, JH], F32, kind="ExternalOutput")
    gbi_o = nc.dram_tensor("gbi_o", [RPC], F32, kind="ExternalOutput")

    with tile.TileContext(nc) as tc:
        with (
            tc.tile_pool(name="sing", bufs=1) as sing,
            tc.tile_pool(name="w1p", bufs=3) as w1p,
            tc.tile_pool(name="gwp", bufs=4) as gwp,
            tc.tile_pool(name="big", bufs=1) as big,
            tc.tile_pool(name="psum", bufs=1, space="PSUM") as psum,
            tc.tile_pool(name="dram", bufs=1, space="DRAM") as dram,
        ):
            # ---- x broadcast (pre-replicated on host) ----
            x_bc = sing.tile([128, DIN], F32)
            nc.sync.dma_start(x_bc[:, :], xbc_d.ap())

            # ---- fc1 forward: z1[r] = W1[r,:] . x, rows r = 5p + n ----
            z1sb = sing.tile([128, NT], F32)
            for n in range(NT):
                w1t = w1p.tile([128, DIN], F32, tag="w1t")
                nc.sync.dma_start(w1t[:, :], w1_d.ap()[n])
                scr = gwp.tile([128, DIN], F32, tag="gw")
                nc.vector.scalar_tensor_tensor(
                    scr[:, :],
                    w1t[:, :],
                    1.0,
                    x_bc[:, :],
                    op0=ALU.mult,
                    op1=ALU.mult,
                    accum_out=z1sb[:, n : n + 1],
                )

            # ---- small input loads ----
            b1p = sing.tile([128, NT], F32)
            nc.sync.dma_start(b1p[:, :], b1_d.ap())
            w2_sb = sing.tile([RPC, W * W], F32)
            nc.sync.dma_start(w2_sb[:, :], w2_d.ap())
            w2t_sb = sing.tile([RPC, W * W], F32)
            nc.sync.dma_start(w2t_sb[:, :], w2t_d.ap())
            b2_sb = sing.tile([RPC, W], F32)
            nc.sync.dma_start(b2_sb[:, :], b2_d.ap())
            bii_sb = sing.tile([RPC, 1], F32)
            nc.sync.dma_start(bii_sb[:, :], bii_d.ap())
            wio_sb = sing.tile([RPC, W], F32)
            nc.sync.dma_start(wio_sb[:, :], wio_d.ap())
            wdh_sb = sing.tile([RPC, 1], F32)
            nc.sync.dma_start(wdh_sb[:, :], wdh_d.ap())
            mdh_sb = sing.tile([RPC, 1], F32)
            nc.sync.dma_start(mdh_sb[:, :], mdh_d.ap())
            pred_sb = sing.tile([RPC, 1], F32)
            nc.sync.dma_start(pred_sb[:, :], pred_d.ap())
            s_sb = sing.tile([128, RPC], F32)
            nc.sync.dma_start(s_sb[:, :], s_d.ap())
            ones_sb = sing.tile([RPC, 1], F32)
            nc.vector.memset(ones_sb[:, :], 1.0)
            # featb rows 0:64 = F.flat[0:JH]; rows 64:128 = F.flat[JH:HW]++hidden.
            # hidden part has no AG dependency, prefetch early.
            featb = sing.tile([128, JH], F32)
            nc.sync.dma_start(
                featb[RPC:128, HW - JH : JH],
                hid_d.ap().unsqueeze(0).broadcast_to([RPC, H]),
            )

            # ---- fc2 forward ----
            zb = sing.tile([128, NT], F32)
            nc.vector.tensor_add(zb[:, :], z1sb[:, :], b1p[:, :])
            f1sb = sing.tile([128, NT], F32)
            nc.vector.tensor_scalar_max(f1sb[:, :], zb[:, :], 0.0)
            # repartition [128,5] (r=5p+n) -> [64,10] (r=10h+w); same linear order
            f1hw = sing.tile([RPC, W], F32)
            nc.sync.dma_start(f1hw[:, :], f1sb[:, :])
            drelu1 = sing.tile([RPC, W], F32)
            nc.vector.tensor_scalar(
                drelu1[:, :], f1hw[:, :], 0.0, None, ALU.is_gt
            )
            f1_bv = f1hw[:, :].unsqueeze(1).broadcast_to([RPC, W, W])
            prod2 = sing.tile([RPC, W * W], F32)
            p2v = prod2[:, :].rearrange("h (v w) -> h v w", w=W)
            nc.vector.tensor_mul(
                p2v, w2_sb[:, :].rearrange("h (v w) -> h v w", w=W), f1_bv
            )
            z2 = sing.tile([RPC, W], F32)
            nc.vector.reduce_sum(z2[:, :], p2v, axis=mybir.AxisListType.X)
            z2b = sing.tile([RPC, W], F32)
            nc.vector.tensor_add(z2b[:, :], z2[:, :], b2_sb[:, :])
            f2 = sing.tile([RPC, W], F32)
            nc.vector.tensor_scalar_max(f2[:, :], z2b[:, :], 0.0)

            # ---- t-independent backward factors (run during the AG bubble) ----
            drelu2 = sing.tile([RPC, W], F32)
            nc.vector.tensor_scalar(drelu2[:, :], f2[:, :], 0.0, None, ALU.is_gt)
            wd2 = sing.tile([RPC, W], F32)  # gz2 = t * wd2
            nc.vector.tensor_mul(wd2[:, :], wio_sb[:, :], drelu2[:, :])
            prodt = sing.tile([RPC, W * W], F32)
            ptv = prodt[:, :].rearrange("h (w v) -> h w v", v=W)
            nc.vector.tensor_mul(
                ptv,
                wd2[:, :].unsqueeze(1).broadcast_to([RPC, W, W]),
                w2t_sb[:, :].rearrange("h (w v) -> h w v", v=W),
            )
            pf1 = sing.tile([RPC, W], F32)  # gf1 = t * pf1
            nc.vector.reduce_sum(pf1[:, :], ptv, axis=mybir.AxisListType.X)
            pz1 = sing.tile([RPC, W], F32)  # gz1 = t * pz1
            nc.vector.tensor_mul(pz1[:, :], pf1[:, :], drelu1[:, :])
            # pz1 repartition [64,10] -> [128,5] before t arrives
            pz1p = sing.tile([128, NT], F32)
            nc.sync.dma_start(pz1p[:, :], pz1[:, :])
            pw2 = sing.tile([RPC, W * W], F32)  # gW2 = t * pw2
            nc.vector.tensor_mul(
                pw2[:, :].rearrange("h (v w) -> h v w", w=W),
                wd2[:, :].unsqueeze(2).broadcast_to([RPC, W, W]),
                f1_bv,
            )
            wdm = sing.tile([RPC, 1], F32)  # ghid = t * wdm
            nc.vector.tensor_mul(wdm[:, :], wdh_sb[:, :], mdh_sb[:, :])

            # ---- AllGather F ----
            f_in = dram.tile([RPC, W], F32)
            nc.sync.dma_start(f_in[:, :], f2[:, :])
            f_out = dram.tile([H, W], F32, addr_space="Shared")
            nc.gpsimd.collective_compute(
                "AllGather",
                ALU.bypass,
                replica_groups=[list(range(NCORES))],
                ins=[f_in[:, :].opt()],
                outs=[f_out[:, :].opt()],
            )

            # ---- i-layer weights: loaded during the AG bubble ----
            wi_sb = big.tile([128, JH], F32, tag="bigA")
            nc.sync.dma_start(wi_sb[:, :], wi_d.ap())
            mask_sb = big.tile([128, JH], F32, tag="mask")
            nc.sync.dma_start(mask_sb[:, :], mask_d.ap())
            wm_sb = big.tile([128, JH], F32, tag="bigB")
            nc.vector.tensor_mul(wm_sb[:, :], wi_sb[:, :], mask_sb[:, :])

            # ---- feat broadcast from AG output (4 queue-parallel DMAs) ----
            f_fl = f_out[:, :].rearrange("a b -> (a b)")
            half = RPC // 2
            for q in range(2):
                nc.sync.dma_start(
                    featb[q * half : (q + 1) * half, :],
                    f_fl[0:JH].unsqueeze(0).broadcast_to([half, JH]),
                )
                nc.sync.dma_start(
                    featb[RPC + q * half : RPC + (q + 1) * half, 0 : HW - JH],
                    f_fl[JH:HW].unsqueeze(0).broadcast_to([half, HW - JH]),
                )

            # ---- u = (Wi*mask) @ feat ; partition p = r + 64s ----
            pu = sing.tile([128, 1], F32)
            scr2 = gwp.tile([128, DIN], F32, tag="gw")
            nc.vector.scalar_tensor_tensor(
                scr2[:, :JH],
                wm_sb[:, :],
                1.0,
                featb[:, :],
                op0=ALU.mult,
                op1=ALU.mult,
                accum_out=pu[:, :],
            )
            u_ps = psum.tile([RPC, 1], F32)
            nc.tensor.matmul(u_ps[:, :], s_sb[:, :], pu[:, :], start=True, stop=True)
            h_sb = sing.tile([RPC, 1], F32)
            nc.scalar.activation(h_sb[:, :], u_ps[:, :], ACTF.Tanh, bias=bii_sb[:, 0:1])
            nc.gpsimd.dma_start(ht_o.ap(), h_sb[:, :])

            # ---- t = 1 - h^2 ----
            hsq = sing.tile([RPC, 1], F32)
            nc.vector.tensor_mul(hsq[:, :], h_sb[:, :], h_sb[:, :])
            t_sb = sing.tile([RPC, 1], F32)
            nc.vector.tensor_scalar(t_sb[:, :], hsq[:, :], -1.0, 1.0, ALU.mult, ALU.add)
            nc.gpsimd.dma_start(gbi_o.ap(), t_sb[:, :])

            # ---- post-t critical path: gz1 -> gz1p -> gW1 ACTs ----
            gz1 = sing.tile([RPC, W], F32)
            nc.vector.tensor_scalar_mul(gz1[:, :], pz1[:, :], t_sb[:, 0:1])
            gz1p = sing.tile([128, NT], F32)
            nc.sync.dma_start(gz1p[:, :], gz1[:, :])
            for n in range(NT):
                gwt = gwp.tile([128, DIN], F32, tag="gw")
                nc.scalar.activation(
                    gwt[:, :], x_bc[:, :], ACTF.Copy, scale=gz1p[:, n : n + 1]
                )
                nc.sync.dma_start(gw1_o.ap()[n], gwt[:, :])

            # ---- remaining small outputs (gpsimd DMAs; off critical path) ----
            nc.gpsimd.dma_start(gb1_o.ap(), gz1[:, :])
            t_il = sing.tile([128, 1], F32)
            nc.vector.tensor_copy(t_il[0:RPC, :], t_sb[:, :])
            nc.gpsimd.dma_start(t_il[RPC:128, :], t_sb[:, :])

            gz2 = sing.tile([RPC, W], F32)
            nc.vector.tensor_scalar_mul(gz2[:, :], wd2[:, :], t_sb[:, 0:1])
            nc.gpsimd.dma_start(gb2_o.ap(), gz2[:, :])
            gw2f = sing.tile([RPC, W * W], F32)
            nc.vector.tensor_scalar_mul(gw2f[:, :], pw2[:, :], t_sb[:, 0:1])
            nc.gpsimd.dma_start(gw2_o.ap(), gw2f[:, :])
            gh = sing.tile([RPC, 1], F32)
            nc.vector.tensor_scalar_mul(gh[:, :], wdm[:, :], t_sb[:, 0:1])
            nc.gpsimd.dma_start(ghid_o.ap(), gh[:, :])

            # ---- y partial = pred_c . h_t ----
            yp = sing.tile([RPC, 1], F32)
            nc.vector.tensor_mul(yp[:, :], pred_sb[:, :], h_sb[:, :])
            y_ps = psum.tile([1, 1], F32)
            nc.tensor.matmul(y_ps[:, :], yp[:, :], ones_sb[:, :], start=True, stop=True)
            y_sb = sing.tile([1, 1], F32)
            nc.vector.tensor_copy(y_sb[:, :], y_ps[:, :])
            nc.gpsimd.dma_start(y_o.ap(), y_sb[:, :])

            # ---- gWi = t * mask * feat (single fused op, off critical path) ----
            gwi_sb = big.tile([128, JH], F32, tag="bigA")
            nc.vector.scalar_tensor_tensor(
                gwi_sb[:, :], mask_sb[:, :], t_il[:, 0:1], featb[:, :],
                op0=ALU.mult, op1=ALU.mult,
            )
            nc.sync.dma_start(gwi_o.ap(), gwi_sb[:, :])

    nc.compile()
    return nc


def shard_inputs(inputs):
    """Full inputs -> per-core in_maps (reshapes/slices/gathers only)."""
    f = lambda a: np.ascontiguousarray(np.asarray(a), dtype=np.float32)
    x = f(inputs["x"])
    hidden = f(inputs["hidden_state"])
    pred = f(inputs["prediction_params"])
    W1 = f(inputs["W1"])
    b1 = f(inputs["b1"])
    W2 = f(inputs["W2"])
    b2 = f(inputs["b2"])
    Wi = f(inputs["Wi"])
    bi = f(inputs["bi"])
    mask = f(inputs["mask"])

    s_fold = np.zeros((128, RPC), np.float32)
    s_fold[np.arange(RPC), np.arange(RPC)] = 1.0
    s_fold[np.arange(RPC) + RPC, np.arange(RPC)] = 1.0
    xbc = np.ascontiguousarray(np.broadcast_to(x, (128, DIN)))

    in_maps = []
    for c in range(NCORES):
        R = slice(RPC * c, RPC * (c + 1))
        rows = np.arange(RPC * c, RPC * (c + 1))
        w1c = W1[R].reshape(FR, DIN)
        m = {
            "xbc_in": xbc,
            "hid_in": hidden,
            "w1_il": np.ascontiguousarray(
                w1c.reshape(128, NT, DIN).transpose(1, 0, 2)
            ),
            "b1_il": np.ascontiguousarray(b1[R].reshape(128, NT)),
            "w2_in": np.ascontiguousarray(W2[R].reshape(RPC, W * W)),
            "w2t_in": np.ascontiguousarray(
                W2[R].transpose(0, 2, 1).reshape(RPC, W * W)
            ),
            "b2_in": np.ascontiguousarray(b2[R]),
            "wi_rc": np.ascontiguousarray(
                Wi[R].reshape(RPC, 2, JH).transpose(1, 0, 2).reshape(128, JH)
            ),
            "mask_rc": np.ascontiguousarray(
                mask[R].reshape(RPC, 2, JH).transpose(1, 0, 2).reshape(128, JH)
            ),
            "bi_c": np.ascontiguousarray(bi[R].reshape(RPC, 1)),
            "wi_own": np.ascontiguousarray(
                Wi[rows[:, None], W * rows[:, None] + np.arange(W)[None, :]]
            ),
            "wi_dh": np.ascontiguousarray(Wi[rows, HW + rows].reshape(RPC, 1)),
            "mask_dh": np.ascontiguousarray(mask[rows, HW + rows].reshape(RPC, 1)),
            "pred_c": np.ascontiguousarray(pred[R].reshape(RPC, 1)),
            "s_fold": s_fold,
        }
        in_maps.append(m)
    return in_maps


def assemble_outputs(rs):
    """Per-core result dicts -> full output tuple (reference order)."""
    cat = lambda k: np.concatenate([r[k] for r in rs], axis=0)
    y = np.float32(sum(float(r["y_o"][0, 0]) for r in rs))
    h_t = cat("ht_o")
    ghid = cat("ghid_o")
    gW1 = np.concatenate(
        [
            r["gw1_o"].transpose(1, 0, 2).reshape(FR, DIN).reshape(RPC, W, DIN)
            for r in rs
        ],
        axis=0,
    )
    gb1 = cat("gb1_o")
    gW2 = np.concatenate([r["gw2_o"].reshape(RPC, W, W) for r in rs], axis=0)
    gb2 = cat("gb2_o")
    gWi = np.concatenate(
        [
            r["gwi_o"].reshape(2, RPC, JH).transpose(1, 0, 2).reshape(RPC, DTOT)
            for r in rs
        ],
        axis=0,
    )
    gbi = cat("gbi_o")
    return (
        np.asarray(y, np.float32),
        h_t, ghid, gW1, gb1, gW2, gb2, gWi, gbi,
    )


_NC_CACHE = None


def kernel(**inputs):
    global _NC_CACHE, LAST_RESULTS
    if _NC_CACHE is None:
        _NC_CACHE = build_program()
    nc = _NC_CACHE
    in_maps = shard_inputs(inputs)
    tcs = os.environ.get("KERNEL_TRACE_CORES")
    res = run_bass_kernel_spmd(
        nc,
        in_maps,
        core_ids=list(range(NCORES)),
        tmpdir=os.environ.get("KERNEL_TRACE_DIR"),
        trace_cores=[int(c) for c in tcs.split(",")] if tcs else None,
    )
    LAST_RESULTS = res
    return assemble_outputs(res.results)
